# revision 8
# baseline (speedup 1.0000x reference)
# Self-contained Trainium2 Bass kernel for nn_Classifier_71768903516861.
# Data-parallel over batch on 8 NeuronCores; tiny AllGather of exp(pre_output)
# mid-kernel (the torch-reshape quirk in `node` mixes global batch indices).
import sys

sys.path.insert(0, "/opt/trn_rl_repo")

from contextlib import ExitStack

import numpy as np

import concourse.bass as bass
import concourse.mybir as mybir
import concourse.tile as tile
from concourse.bass_utils import run_bass_kernel_spmd

F32 = mybir.dt.float32
AF = mybir.ActivationFunctionType

N_CORES = 8
M, CLS, K, DL, DC, DO, BD = 87, 65, 128, 128, 128, 2048, 256
B, H, W = 256, 7, 7
HW = H * W          # 49
BL = B // N_CORES   # 32 batches per core
NCT = DO // 128     # 16 channel tiles
NW = BL * HW        # 1568 free columns of x per core
NBG = 4             # batch groups of 8 for conv psum tiling
BGW = NW // NBG     # 392


def split_multi_waits(nc):
    """This walrus encodes exactly one sync-wait per instruction; hoist extras
    into standalone single-wait EventSemaphore ops on the same engine."""
    n = 0
    for func in nc.m.functions:
        for block in func.blocks:
            out = []
            for inst in list(block.instructions):
                si = inst.sync_info
                ow = list(si.on_wait) if si is not None and si.on_wait else []
                if len(ow) > 1:
                    for j, w in enumerate(ow[:-1]):
                        es = mybir.InstEventSemaphore(
                            name=f"{inst.name}_hw{j}",
                            engine=inst.engine,
                            ins=[],
                            outs=[],
                            sync_info=mybir.SyncInfo(on_wait=[w], on_update=[]),
                        )
                        out.append(es)
                        n += 1
                    inst.sync_info = mybir.SyncInfo(
                        on_wait=[ow[-1]], on_update=list(si.on_update)
                    )
                out.append(inst)
            block.instructions = out
    return n


def _build_program():
    nc = bass.Bass("TRN2", target_bir_lowering=False, debug=False,
                   num_devices=N_CORES)

    def inp(name, shape):
        return nc.dram_tensor(name, list(shape), F32, kind="ExternalInput").ap()

    def outp(name, shape):
        return nc.dram_tensor(name, list(shape), F32, kind="ExternalOutput").ap()

    d_x = inp("x_t", (DO, NW))            # per-core x, [c, b*hw]
    d_wo1 = inp("wo1", (4 * DL, DO))      # [512, 2048]
    d_wo2t = inp("wo2t", (4 * DL, DL))    # Wo2.T [512, 128]
    d_bo1 = inp("bo1c", (4 * DL, 1))
    d_bo2 = inp("bo2c", (DL, 1))
    d_wbt = inp("wbt", (DO, BD))          # Wb.T [2048, 256]
    d_bb = inp("bb_row", (1, BD))
    d_wfct = inp("wfct", (BD, CLS))       # Wfc.T [256, 65]
    d_wfc2t = inp("wfc2t", (BD, CLS))
    d_wat = inp("wat", (DL, M))           # Wa.T [128, 87]
    d_ba = inp("ba_col", (M, 1))
    d_wpst = inp("wpst", (DL, DC))        # Wps.T [128, 128]
    d_bps = inp("bps_col", (DC, 1))
    d_wgt = inp("wgt", (DC + K, DC))      # Wg.T [256, 128]
    d_bg = inp("bg_row", (1, DC))
    d_wspt = inp("wspt", (DC, DL))        # Wsp.T [128, 128]
    d_bsp = inp("bsp_row", (1, DL))
    d_ws1 = inp("ws1_row", (1, DC))       # Ws[0,:128]
    d_nodep = inp("node_p", (K, M))
    d_edge = inp("edge", (M, M))
    d_wox2b = inp("wox2b", (DO, 4 * DL + 1))  # [Wox2 | box2] [2048, 513]
    d_box1 = inp("box1c", (4 * DL, 1))
    d_wox1 = inp("wox1", (4 * DL, DL))    # [512, 128]
    d_id = inp("id128", (128, 128))

    o_feat = outp("features", (BL, BD))
    o_out2 = outp("outputs2", (BL, CLS))
    o_soft = outp("softmax_outputs", (BL, CLS))
    o_pre = outp("pre_output", (BL, CLS))
    o_featp = outp("features_p", (BL, BD))

    with tile.TileContext(nc) as tc, ExitStack() as ctx:
        per = ctx.enter_context(tc.tile_pool(name="persist", bufs=1))
        dram = ctx.enter_context(tc.tile_pool(name="dram", bufs=1, space="DRAM"))
        pp_pr = ctx.enter_context(tc.tile_pool(name="pp_pr", bufs=2, space="PSUM"))

        # ---- constants & small loads ----
        idt = per.tile([128, 128], F32, name="idt")
        nc.sync.dma_start(idt[:], d_id[:])
        ones_col = per.tile([128, 1], F32, name="ones_col")
        nc.vector.memset(ones_col[:], 1.0)
        ones_row = per.tile([1, 128], F32, name="ones_row")
        nc.vector.memset(ones_row[:], 1.0)

        edge_t = per.tile([M, M], F32, name="edge_t")
        nc.sync.dma_start(edge_t[:], d_edge[:])
        nodep_t = per.tile([K, M], F32, name="nodep_t")
        nc.sync.dma_start(nodep_t[:], d_nodep[:])
        bb_row = per.tile([1, BD], F32, name="bb_row_t")
        nc.sync.dma_start(bb_row[:], d_bb[:])
        bg_row = per.tile([1, DC], F32, name="bg_row_t")
        nc.sync.dma_start(bg_row[:], d_bg[:])
        bsp_row = per.tile([1, DL], F32, name="bsp_row_t")
        nc.sync.dma_start(bsp_row[:], d_bsp[:])
        ws1_row = per.tile([1, DC], F32, name="ws1_row_t")
        nc.sync.dma_start(ws1_row[:], d_ws1[:])
        ba_col = per.tile([M, 1], F32, name="ba_col_t")
        nc.sync.dma_start(ba_col[:], d_ba[:])
        bps_col = per.tile([DC, 1], F32, name="bps_col_t")
        nc.sync.dma_start(bps_col[:], d_bps[:])
        bo1c = per.tile([128, 4, 1], F32, name="bo1c_t")
        nc.sync.dma_start(bo1c[:], d_bo1.rearrange("(t p) c -> p t c", p=128))
        bo2c = per.tile([DL, 1], F32, name="bo2c_t")
        nc.sync.dma_start(bo2c[:], d_bo2[:])
        box1c = per.tile([128, 4, 1], F32, name="box1c_t")
        nc.sync.dma_start(box1c[:], d_box1.rearrange("(t p) c -> p t c", p=128))
        wat_t = per.tile([DL, M], F32, name="wat_t")
        nc.sync.dma_start(wat_t[:], d_wat[:])
        wpst_t = per.tile([DL, DC], F32, name="wpst_t")
        nc.sync.dma_start(wpst_t[:], d_wpst[:])
        wgt_t = per.tile([128, 2, DC], F32, name="wgt_t")
        nc.sync.dma_start(wgt_t[:], d_wgt.rearrange("(t p) c -> p t c", p=128))
        wspt_t = per.tile([DC, DL], F32, name="wspt_t")
        nc.sync.dma_start(wspt_t[:], d_wspt[:])
        wfct_t = per.tile([128, 2, CLS], F32, name="wfct_t")
        nc.sync.dma_start(wfct_t[:], d_wfct.rearrange("(t p) c -> p t c", p=128))
        wfc2t_t = per.tile([128, 2, CLS], F32, name="wfc2t_t")
        nc.sync.dma_start(wfc2t_t[:], d_wfc2t.rearrange("(t p) c -> p t c", p=128))
        wo2t_t = per.tile([128, 4, DL], F32, name="wo2t_t")
        nc.sync.dma_start(wo2t_t[:], d_wo2t.rearrange("(t p) c -> p t c", p=128))
        wbt_t = per.tile([128, NCT, BD], F32, name="wbt_t")
        nc.sync.dma_start(wbt_t[:], d_wbt.rearrange("(t p) c -> p t c", p=128))

        # Ws1 broadcast over 87 partitions (for per-elem t_node via DVE)
        ws1b_ps = pp_pr.tile([M, DC], F32, name="ws1b_ps", tag="pr")
        nc.tensor.matmul(ws1b_ps[:], ones_row[0:1, 0:M], ws1_row[:],
                         start=True, stop=True)
        ws1b = per.tile([M, DC], F32, name="ws1b")
        nc.scalar.activation(ws1b[:], ws1b_ps[:], AF.Identity)

        # ---- PageRank prep: Q^T from edge via doubling ----
        rs = per.tile([M, 1], F32, name="rs")
        nc.vector.reduce_sum(rs[:], edge_t[:], axis=mybir.AxisListType.X)
        rr = per.tile([M, 1], F32, name="rr")
        nc.vector.reciprocal(rr[:], rs[:])
        # P^T = 0.85 * edge / rowsum  (edge symmetric)
        pt_t = per.tile([M, M], F32, name="pt_t")
        nc.vector.tensor_scalar(pt_t[:], edge_t[:], rr[:], 0.85,
                                op0=mybir.AluOpType.mult,
                                op1=mybir.AluOpType.mult)
        p_ps = pp_pr.tile([M, M], F32, name="p_ps", tag="pr")
        nc.tensor.transpose(p_ps[:], pt_t[:], idt[0:M, 0:M])
        p_t = per.tile([M, M], F32, name="p_t")
        nc.scalar.activation(p_t[:], p_ps[:], AF.Identity)

        # edge_norm = Dinv edge Dinv, Dinv = rowsum^-1/2
        sqd = per.tile([M, 1], F32, name="sqd")
        nc.scalar.activation(sqd[:], rs[:], AF.Sqrt)
        dinv = per.tile([M, 1], F32, name="dinv")
        nc.vector.reciprocal(dinv[:], sqd[:])
        a1 = per.tile([M, M], F32, name="a1")
        nc.vector.tensor_scalar_mul(a1[:], edge_t[:], dinv[:])
        a1t_ps = pp_pr.tile([M, M], F32, name="a1t_ps", tag="pr")
        nc.tensor.transpose(a1t_ps[:], a1[:], idt[0:M, 0:M])
        a1t = per.tile([M, M], F32, name="a1t")
        nc.scalar.activation(a1t[:], a1t_ps[:], AF.Identity)
        en_t = per.tile([M, M], F32, name="en_t")
        nc.vector.tensor_scalar_mul(en_t[:], a1t[:], dinv[:])

        # doubling state: X=P^a, Xt=(P^a)^T, S=I+P+...+P^(a-1)
        x_c = per.tile([M, M], F32, name="x_cur")
        nc.vector.tensor_copy(x_c[:], p_t[:])
        xt_c = per.tile([M, M], F32, name="xt_cur")
        nc.vector.tensor_copy(xt_c[:], pt_t[:])
        s_c = per.tile([M, M], F32, name="s_cur")
        nc.vector.tensor_copy(s_c[:], idt[0:M, 0:M])

        def pr_double(i):
            x2_ps = pp_pr.tile([M, M], F32, name="x2_ps", tag="pr")
            nc.tensor.matmul(x2_ps[:], xt_c[:], x_c[:], start=True, stop=True)
            xt2_ps = pp_pr.tile([M, M], F32, name="xt2_ps", tag="pr")
            nc.tensor.matmul(xt2_ps[:], x_c[:], xt_c[:], start=True, stop=True)
            s2_ps = pp_pr.tile([M, M], F32, name="s2_ps", tag="pr")
            nc.tensor.matmul(s2_ps[:], xt_c[:], s_c[:], start=True, stop=True)
            nc.vector.tensor_add(s_c[:], s_c[:], s2_ps[:])
            nc.scalar.activation(x_c[:], x2_ps[:], AF.Identity)
            nc.scalar.activation(xt_c[:], xt2_ps[:], AF.Identity)

        def pr_inc(i):
            nc.vector.tensor_add(s_c[:], s_c[:], x_c[:])
            x2_ps = pp_pr.tile([M, M], F32, name="xi_ps", tag="pr")
            nc.tensor.matmul(x2_ps[:], xt_c[:], p_t[:], start=True, stop=True)
            xt2_ps = pp_pr.tile([M, M], F32, name="xti_ps", tag="pr")
            nc.tensor.matmul(xt2_ps[:], p_t[:], xt_c[:], start=True, stop=True)
            nc.scalar.activation(x_c[:], x2_ps[:], AF.Identity)
            nc.scalar.activation(xt_c[:], xt2_ps[:], AF.Identity)

        # a: 1 ->2 ->3 ->6 ->12 ->24 ->25 ->50 ->100
        pr_double(0); pr_inc(1); pr_double(2); pr_double(3)
        pr_double(4); pr_inc(5); pr_double(6); pr_double(7)

        # Q = X + 0.15*S ; Qt = Q^T
        q_t = per.tile([M, M], F32, name="q_t")
        nc.vector.scalar_tensor_tensor(
            out=q_t[:], in0=s_c[:], scalar=0.15, in1=x_c[:],
            op0=mybir.AluOpType.mult, op1=mybir.AluOpType.add)
        qt_ps = pp_pr.tile([M, M], F32, name="qt_ps", tag="pr")
        nc.tensor.transpose(qt_ps[:], q_t[:], idt[0:M, 0:M])
        qt_t = per.tile([M, M], F32, name="qt_t")
        nc.scalar.activation(qt_t[:], qt_ps[:], AF.Identity)

        # ---- phase A: W_l prep + conv + pooling + output-side W prep ----
        wlt_all = per.tile([128, NCT, 128], F32, name="wlt_all")
        xl_all = per.tile([128, NW], F32, name="xl_all")
        xsum_all = per.tile([128, NCT, BL], F32, name="xsum_all")
        b_l = per.tile([DL, 1], F32, name="b_l")
        tmpt_all = per.tile([128, 4, BD], F32, name="tmpt_all")
        tmpt_x = per.tile([1, BD], F32, name="tmpt_x")
        w2t_t = per.tile([DL, BD], F32, name="w2t_t")
        b2_row = per.tile([1, BD], F32, name="b2_row")

        with ExitStack() as ctx_a:
            pp_prep = ctx_a.enter_context(
                tc.tile_pool(name="pp_prep", bufs=2, space="PSUM"))
            pp_conv = ctx_a.enter_context(
                tc.tile_pool(name="pp_conv", bufs=1, space="PSUM"))
            sb_w = ctx_a.enter_context(tc.tile_pool(name="sb_w", bufs=1))
            sb_x = ctx_a.enter_context(tc.tile_pool(name="sb_x", bufs=3))

            # W_l^T tiles: WlT[ct] = sum_h Wo1[h, ct-block]^T @ Wo2^T
            for ct in range(NCT):
                wo1_blk = sb_w.tile([128, 4, 128], F32, name="wo1_blk", bufs=3)
                nc.sync.dma_start(
                    wo1_blk[:],
                    d_wo1.rearrange("(t p) c -> p t c", p=128)[
                        :, :, ct * 128:(ct + 1) * 128])
                wl_ps = pp_prep.tile([128, 128], F32, name="wl_ps", tag="prep")
                for h in range(4):
                    nc.tensor.matmul(wl_ps[:], wo1_blk[:, h, :],
                                     wo2t_t[:, h, :],
                                     start=(h == 0), stop=(h == 3))
                nc.scalar.activation(wlt_all[:, ct, :], wl_ps[:], AF.Identity)

            # b_l = Wo2 @ bo1 + bo2
            bl_ps = pp_prep.tile([DL, 1], F32, name="bl_ps", tag="prep")
            for h in range(4):
                nc.tensor.matmul(bl_ps[:], wo2t_t[:, h, :], bo1c[:, h, :],
                                 start=(h == 0), stop=(h == 3))
            nc.vector.tensor_add(b_l[:], bl_ps[:], bo2c[:])

            # conv + pooling, ct-outer, bg-inner
            conv_ps = [pp_conv.tile([128, BGW], F32, name=f"conv_ps{bg}")
                       for bg in range(NBG)]
            for ct in range(NCT):
                x_ct = sb_x.tile([128, NW], F32, name="x_ct")
                nc.sync.dma_start(x_ct[:], d_x[ct * 128:(ct + 1) * 128, :])
                for bg in range(NBG):
                    nc.tensor.matmul(
                        conv_ps[bg][:], wlt_all[:, ct, :],
                        x_ct[:, bg * BGW:(bg + 1) * BGW],
                        start=(ct == 0), stop=(ct == NCT - 1))
                nc.vector.reduce_sum(
                    xsum_all[:, ct, :],
                    x_ct.rearrange("p (b w) -> p b w", w=HW),
                    axis=mybir.AxisListType.X)
            for bg in range(NBG):
                nc.scalar.activation(xl_all[:, bg * BGW:(bg + 1) * BGW],
                                     conv_ps[bg][:], AF.Identity,
                                     bias=b_l[:])

            # output-side prep: tmpT = [Wox2|box2]^T @ Wb^T
            wox2_blks = []
            for ct in range(NCT):
                blk = sb_w.tile([128, 4 * DL + 1], F32,
                                name=f"wox2_blk{ct}", bufs=1)
                nc.sync.dma_start(
                    blk[:], d_wox2b[ct * 128:(ct + 1) * 128, :])
                wox2_blks.append(blk)
            for mt in range(4):
                tm_ps = pp_prep.tile([128, BD], F32, name="tm_ps", tag="prep")
                for ct in range(NCT):
                    nc.tensor.matmul(
                        tm_ps[:], wox2_blks[ct][:, mt * 128:(mt + 1) * 128],
                        wbt_t[:, ct, :], start=(ct == 0), stop=(ct == NCT - 1))
                nc.scalar.activation(tmpt_all[:, mt, :], tm_ps[:], AF.Identity)
            tx_ps = pp_prep.tile([1, BD], F32, name="tx_ps", tag="prep")
            for ct in range(NCT):
                nc.tensor.matmul(tx_ps[:],
                                 wox2_blks[ct][:, 4 * DL:4 * DL + 1],
                                 wbt_t[:, ct, :],
                                 start=(ct == 0), stop=(ct == NCT - 1))
            nc.scalar.activation(tmpt_x[:], tx_ps[:], AF.Identity)

            # W2^T = Wox1^T @ tmpT ; b2 = box1^T @ tmpT + (Wb box2)^T + bb
            wox1_t = sb_w.tile([128, 4, DL], F32, name="wox1_t")
            nc.sync.dma_start(wox1_t[:],
                              d_wox1.rearrange("(t p) c -> p t c", p=128))
            w2_ps = pp_prep.tile([DL, BD], F32, name="w2_ps", tag="prep")
            for h in range(4):
                nc.tensor.matmul(w2_ps[:], wox1_t[:, h, :], tmpt_all[:, h, :],
                                 start=(h == 0), stop=(h == 3))
            nc.scalar.activation(w2t_t[:], w2_ps[:], AF.Identity)
            b2_ps = pp_prep.tile([1, BD], F32, name="b2_ps", tag="prep")
            for h in range(4):
                nc.tensor.matmul(b2_ps[:], box1c[:, h, :], tmpt_all[:, h, :],
                                 start=(h == 0), stop=(h == 3))
            b2a = per.tile([1, BD], F32, name="b2a")
            nc.vector.tensor_add(b2a[:], b2_ps[:], tmpt_x[:])
            nc.vector.tensor_add(b2_row[:], b2a[:], bb_row[0:1, :])

        # ---- phase B: mid chains ----
        a_all = per.tile([M, NW], F32, name="a_all")
        ps_all = per.tile([DC, NW], F32, name="ps_all")
        m1_all = per.tile([DL, BL * M], F32, name="m1_all")
        featp_t = per.tile([BL, BD], F32, name="featp_t")
        fpt2 = per.tile([128, 2, BL], F32, name="fpt2")
        preo_t = per.tile([BL, CLS], F32, name="preo_t")
        e_l = per.tile([CLS, BL], F32, name="e_l")
        # E zero-padded to 87 rows so PE reads at base partition 0
        e_full = per.tile([M, B], F32, name="e_full")
        eT_all = per.tile([M, BL], F32, name="eT_all")
        q_all = per.tile([DL, BL], F32, name="q_all")
        rr_all = per.tile([DL, BL], F32, name="rr_all")

        with ExitStack() as ctx_b:
            pp_mid = ctx_b.enter_context(
                tc.tile_pool(name="pp_mid", bufs=2, space="PSUM"))
            pp_pe = ctx_b.enter_context(
                tc.tile_pool(name="pp_pe", bufs=3, space="PSUM"))
            sb_pe = ctx_b.enter_context(tc.tile_pool(name="sb_pe", bufs=3))

            # a_all = Wa @ x_l + ba ; ps_all = Wps @ x_l + bps
            for bg in range(NBG):
                sl = slice(bg * BGW, (bg + 1) * BGW)
                a_ps = pp_mid.tile([M, BGW], F32, name="a_ps", tag="mid")
                nc.tensor.matmul(a_ps[:], wat_t[:], xl_all[:, sl],
                                 start=True, stop=True)
                nc.scalar.activation(a_all[:, sl], a_ps[:], AF.Identity,
                                     bias=ba_col[:])
                p_ps2 = pp_mid.tile([DC, BGW], F32, name="p_ps2", tag="mid")
                nc.tensor.matmul(p_ps2[:], wpst_t[:], xl_all[:, sl],
                                 start=True, stop=True)
                nc.scalar.activation(ps_all[:, sl], p_ps2[:], AF.Identity,
                                     bias=bps_col[:])

            # per-elem m1 = relu(ps[b] @ aT[b])
            for b in range(BL):
                sl = slice(b * HW, (b + 1) * HW)
                at_ps = pp_pe.tile([HW, M], F32, name="at_ps", tag="pe")
                nc.tensor.transpose(at_ps[:], a_all[:, sl], idt[0:M, 0:M])
                at_sb = sb_pe.tile([HW, M], F32, name="at_sb")
                nc.scalar.activation(at_sb[:], at_ps[:], AF.Identity)
                pst_ps = pp_pe.tile([HW, DC], F32, name="pst_ps", tag="pe")
                nc.tensor.transpose(pst_ps[:], ps_all[:, sl], idt[:])
                pst_sb = sb_pe.tile([HW, DC], F32, name="pst_sb")
                nc.scalar.activation(pst_sb[:], pst_ps[:], AF.Identity)
                m1_ps = pp_pe.tile([DL, M], F32, name="m1_ps", tag="pe")
                nc.tensor.matmul(m1_ps[:], pst_sb[:], at_sb[:],
                                 start=True, stop=True)
                nc.scalar.activation(m1_all[:, b * M:(b + 1) * M], m1_ps[:],
                                     AF.Relu)

            # features_p^T = xsum/49 @ Wb^T + bb  (bias row pre-scaled by 49)
            bb49 = per.tile([1, BD], F32, name="bb49")
            nc.vector.tensor_scalar_mul(bb49[:], bb_row[:], float(HW))
            fp_ps = pp_mid.tile([BL, BD], F32, name="fp_ps", tag="mid")
            for ct in range(NCT):
                nc.tensor.matmul(fp_ps[:], xsum_all[:, ct, :], wbt_t[:, ct, :],
                                 start=(ct == 0), stop=False)
            nc.tensor.matmul(fp_ps[:], ones_row[0:1, 0:BL], bb49[:],
                             start=False, stop=True)
            nc.scalar.activation(featp_t[:], fp_ps[:], AF.Identity,
                                 scale=1.0 / HW)
            nc.sync.dma_start(o_featp[:], featp_t[:])

            # transpose features_p^T -> [256, 32]
            for t in range(2):
                f2_ps = pp_pe.tile([128, BL], F32, name="f2_ps", tag="pe")
                nc.tensor.transpose(f2_ps[:],
                                    featp_t[:, t * 128:(t + 1) * 128],
                                    idt[0:BL, 0:BL])
                nc.scalar.activation(fpt2[:, t, :], f2_ps[:], AF.Identity)

            # pre_output (both layouts) and E = exp(pre_outE)
            po_ps = pp_mid.tile([BL, CLS], F32, name="po_ps", tag="mid")
            for t in range(2):
                nc.tensor.matmul(po_ps[:], fpt2[:, t, :], wfct_t[:, t, :],
                                 start=(t == 0), stop=(t == 1))
            nc.scalar.activation(preo_t[:], po_ps[:], AF.Identity)
            nc.sync.dma_start(o_pre[:], preo_t[:])
            pe_ps = pp_mid.tile([CLS, BL], F32, name="pe_ps", tag="mid")
            for t in range(2):
                nc.tensor.matmul(pe_ps[:], wfct_t[:, t, :], fpt2[:, t, :],
                                 start=(t == 0), stop=(t == 1))
            nc.scalar.activation(e_l[:], pe_ps[:], AF.Exp)

            # AllGather E across cores
            nc.vector.memset(e_full[:], 0.0)
            ag_in = dram.tile([CLS, BL], F32, name="ag_in")
            ag_out = dram.tile([N_CORES, CLS, BL], F32, name="ag_out")
            nc.sync.dma_start(ag_in[:], e_l[:])
            nc.gpsimd.collective_compute(
                "AllGather", mybir.AluOpType.bypass,
                replica_groups=[list(range(N_CORES))],
                ins=[ag_in.opt()], outs=[ag_out.opt()])
            nc.sync.dma_start(
                e_full[M - CLS:M, :].rearrange("p (g b) -> p g b", g=N_CORES),
                ag_out.rearrange("g p b -> p g b"))

            # v_y for all 256 global batches
            srow_ps = pp_mid.tile([1, B], F32, name="srow_ps", tag="mid")
            nc.tensor.matmul(srow_ps[:], ones_col[0:M, :], e_full[:],
                             start=True, stop=True)
            rS = per.tile([1, B], F32, name="rS")
            nc.vector.reciprocal(rS[:], srow_ps[:])
            u_ps = pp_mid.tile([M, B], F32, name="u_ps", tag="mid")
            nc.tensor.matmul(u_ps[:], qt_t[:], e_full[:],
                             start=True, stop=True)
            u_sb = per.tile([M, B], F32, name="u_sb")
            nc.scalar.activation(u_sb[:], u_ps[:], AF.Identity)
            u2 = per.tile([M, B], F32, name="u2")
            nc.scalar.activation(u2[:], u_sb[:], AF.Square)
            ss_ps = pp_mid.tile([1, B], F32, name="ss_ps", tag="mid")
            nc.tensor.matmul(ss_ps[:], ones_col[0:M, :], u2[:],
                             start=True, stop=True)
            nrm = per.tile([1, B], F32, name="nrm")
            nc.scalar.activation(nrm[:], ss_ps[:], AF.Sqrt)
            rn = per.tile([1, B], F32, name="rn")
            nc.vector.reciprocal(rn[:], nrm[:])
            rnb_ps = pp_mid.tile([M, B], F32, name="rnb_ps", tag="mid")
            nc.tensor.matmul(rnb_ps[:], ones_row[0:1, 0:M], rn[:],
                             start=True, stop=True)
            vy = per.tile([M, B], F32, name="vy")
            nc.vector.tensor_mul(vy[:], u_sb[:], rnb_ps[:])
            rsb_ps = pp_mid.tile([M, B], F32, name="rsb_ps", tag="mid")
            nc.tensor.matmul(rsb_ps[:], ones_row[0:1, 0:M], rS[:],
                             start=True, stop=True)
            vinit = per.tile([M, B], F32, name="vinit")
            nc.vector.tensor_mul(vinit[:], e_full[:], rsb_ps[:])
            nc.vector.tensor_add(vy[:], vy[:], vinit[:])

            # node_e/node_o from v_y^T
            node_eo = []
            for t in range(2):
                vt_ps = pp_pe.tile([128, M], F32, name="vt_ps", tag="pe")
                nc.tensor.transpose(vt_ps[:], vy[:, t * 128:(t + 1) * 128],
                                    idt[0:M, 0:M])
                vt_sb = sb_pe.tile([128, M], F32, name="vt_sb",
                                   tag=f"vt{t}")
                nc.scalar.activation(vt_sb[:], vt_ps[:], AF.Identity)
                node_t = per.tile([K, M], F32, name=f"node_{t}")
                nc.vector.tensor_mul(node_t[:], nodep_t[:], vt_sb[:])
                node_eo.append(node_t)

            # ---- per-elem post chain ----
            for b in range(BL):
                node_par = node_eo[b % 2]
                gt_ps = pp_pe.tile([M, DC], F32, name="gt_ps", tag="pe")
                nc.tensor.matmul(gt_ps[:], node_par[:], wgt_t[:, 0, :],
                                 start=True, stop=False)
                nc.tensor.matmul(gt_ps[:], m1_all[:, b * M:(b + 1) * M],
                                 wgt_t[:, 1, :], start=False, stop=False)
                nc.tensor.matmul(gt_ps[:], ones_row[0:1, 0:M], bg_row[:],
                                 start=False, stop=True)
                gt_sb = sb_pe.tile([M, DC], F32, name="gt_sb")
                nc.scalar.activation(gt_sb[:], gt_ps[:], AF.Identity)
                m2t_ps = pp_pe.tile([M, DC], F32, name="m2t_ps", tag="pe")
                nc.tensor.matmul(m2t_ps[:], en_t[:], gt_sb[:],
                                 start=True, stop=True)
                m2t_sb = sb_pe.tile([M, DC], F32, name="m2t_sb")
                nc.scalar.activation(m2t_sb[:], m2t_ps[:], AF.Relu)
                wtn = sb_pe.tile([M, DC], F32, name="wtn")
                nc.vector.tensor_mul(wtn[:], m2t_sb[:], ws1b[:])
                tn = sb_pe.tile([M, 1], F32, name="tn")
                nc.vector.reduce_sum(tn[:], wtn[:], axis=mybir.AxisListType.X)
                nc.scalar.activation(eT_all[:, b:b + 1], tn[:], AF.Exp)
                q_ps = pp_pe.tile([DL, 1], F32, name="q_ps", tag="pe")
                nc.tensor.matmul(q_ps[:], m2t_sb[:], eT_all[:, b:b + 1],
                                 start=True, stop=True)
                nc.scalar.activation(q_all[:, b:b + 1], q_ps[:], AF.Identity)

            # Z, then r' = relu(Wsp@q' + Z*bsp)
            z_ps = pp_mid.tile([1, BL], F32, name="z_ps", tag="mid")
            nc.tensor.matmul(z_ps[:], ones_col[0:M, :], eT_all[:],
                             start=True, stop=True)
            z_row = per.tile([1, BL], F32, name="z_row")
            nc.scalar.activation(z_row[:], z_ps[:], AF.Identity)
            for b in range(BL):
                r_ps = pp_pe.tile([DL, 1], F32, name="r_ps", tag="pe")
                nc.tensor.matmul(r_ps[:], wspt_t[:], q_all[:, b:b + 1],
                                 start=True, stop=False)
                nc.tensor.matmul(r_ps[:], bsp_row[:], z_row[:, b:b + 1],
                                 start=False, stop=True)
                nc.scalar.activation(rr_all[:, b:b + 1], r_ps[:], AF.Relu)

            # 1/Z as per-partition column [32, 1]
            rz_row = per.tile([1, BL], F32, name="rz_row")
            nc.vector.reciprocal(rz_row[:], z_row[:])
            rzc_ps = pp_pe.tile([BL, 1], F32, name="rzc_ps", tag="pe")
            nc.tensor.transpose(rzc_ps[:], rz_row[:], idt[0:1, 0:1])
            rz_col = per.tile([BL, 1], F32, name="rz_col")
            nc.scalar.activation(rz_col[:], rzc_ps[:], AF.Identity)

            # feature_aug^T = (r'^T @ W2^T + Z (x) b2) / Z ; features
            aug_ps = pp_mid.tile([BL, BD], F32, name="aug_ps", tag="mid")
            nc.tensor.matmul(aug_ps[:], rr_all[:], w2t_t[:],
                             start=True, stop=False)
            nc.tensor.matmul(aug_ps[:], z_row[:], b2_row[:],
                             start=False, stop=True)
            aug_t = per.tile([BL, BD], F32, name="aug_t")
            nc.scalar.activation(aug_t[:], aug_ps[:], AF.Identity,
                                 scale=rz_col[:])
            feat_t = per.tile([BL, BD], F32, name="feat_t")
            nc.vector.tensor_add(feat_t[:], featp_t[:], aug_t[:])
            nc.sync.dma_start(o_feat[:], feat_t[:])

            # outputs2 = features @ Wfc2^T ; softmax
            ft2 = per.tile([128, 2, BL], F32, name="ft2")
            for t in range(2):
                ff_ps = pp_pe.tile([128, BL], F32, name="ff_ps", tag="pe")
                nc.tensor.transpose(ff_ps[:], feat_t[:, t * 128:(t + 1) * 128],
                                    idt[0:BL, 0:BL])
                nc.scalar.activation(ft2[:, t, :], ff_ps[:], AF.Identity)
            o2_ps = pp_mid.tile([BL, CLS], F32, name="o2_ps", tag="mid")
            for t in range(2):
                nc.tensor.matmul(o2_ps[:], ft2[:, t, :], wfc2t_t[:, t, :],
                                 start=(t == 0), stop=(t == 1))
            out2_t = per.tile([BL, CLS], F32, name="out2_t")
            nc.scalar.activation(out2_t[:], o2_ps[:], AF.Identity)
            nc.sync.dma_start(o_out2[:], out2_t[:])
            esm = per.tile([BL, CLS], F32, name="esm")
            sums = per.tile([BL, 1], F32, name="sums")
            nc.scalar.activation(esm[:], o2_ps[:], AF.Exp, accum_out=sums[:])
            rsm = per.tile([BL, 1], F32, name="rsm")
            nc.vector.reciprocal(rsm[:], sums[:])
            soft_t = per.tile([BL, CLS], F32, name="soft_t")
            nc.vector.tensor_scalar_mul(soft_t[:], esm[:], rsm[:])
            nc.sync.dma_start(o_soft[:], soft_t[:])

    split_multi_waits(nc)
    return nc


_NC_CACHE = {}


def _get_nc():
    if "nc" not in _NC_CACHE:
        _NC_CACHE["nc"] = _build_program()
    return _NC_CACHE["nc"]


def _host_prep(inputs):
    f = np.float32
    x = np.ascontiguousarray(np.asarray(inputs["x"], f))  # [256, 2048, 7, 7]
    shared = {
        "wo1": np.ascontiguousarray(np.asarray(inputs["Wo1"], f)),
        "wo2t": np.ascontiguousarray(np.asarray(inputs["Wo2"], f).T),
        "bo1c": np.asarray(inputs["bo1"], f).reshape(4 * DL, 1).copy(),
        "bo2c": np.asarray(inputs["bo2"], f).reshape(DL, 1).copy(),
        "wbt": np.ascontiguousarray(np.asarray(inputs["Wb"], f).T),
        "bb_row": np.asarray(inputs["bb"], f).reshape(1, BD).copy(),
        "wfct": np.ascontiguousarray(np.asarray(inputs["Wfc"], f).T),
        "wfc2t": np.ascontiguousarray(np.asarray(inputs["Wfc2"], f).T),
        "wat": np.ascontiguousarray(np.asarray(inputs["Wa"], f).T),
        "ba_col": np.asarray(inputs["ba"], f).reshape(M, 1).copy(),
        "wpst": np.ascontiguousarray(np.asarray(inputs["Wps"], f).T),
        "bps_col": np.asarray(inputs["bps"], f).reshape(DC, 1).copy(),
        "wgt": np.ascontiguousarray(np.asarray(inputs["Wg"], f).T),
        "bg_row": np.asarray(inputs["bg"], f).reshape(1, DC).copy(),
        "wspt": np.ascontiguousarray(np.asarray(inputs["Wsp"], f).T),
        "bsp_row": np.asarray(inputs["bsp"], f).reshape(1, DL).copy(),
        "ws1_row": np.asarray(inputs["Ws"], f)[0, :DC].reshape(1, DC).copy(),
        "node_p": np.ascontiguousarray(np.asarray(inputs["node_p"], f)),
        "edge": np.ascontiguousarray(np.asarray(inputs["edge"], f)),
        "wox2b": np.ascontiguousarray(np.concatenate(
            [np.asarray(inputs["Wox2"], f),
             np.asarray(inputs["box2"], f).reshape(DO, 1)], axis=1)),
        "box1c": np.asarray(inputs["box1"], f).reshape(4 * DL, 1).copy(),
        "wox1": np.ascontiguousarray(np.asarray(inputs["Wox1"], f)),
        "id128": np.eye(128, dtype=f),
    }
    in_maps = []
    for c in range(N_CORES):
        xc = x[c * BL:(c + 1) * BL].reshape(BL, DO, HW)
        xt = np.ascontiguousarray(xc.transpose(1, 0, 2).reshape(DO, NW))
        m = dict(shared)
        m["x_t"] = xt
        in_maps.append(m)
    return in_maps


def kernel(**inputs):
    nc = _get_nc()
    in_maps = _host_prep(inputs)
    res = run_bass_kernel_spmd(nc, in_maps, core_ids=list(range(N_CORES)))
    feats, out2s, softs, pres, featps = [], [], [], [], []
    for c in range(N_CORES):
        r = res.results[c]
        feats.append(r["features"])
        out2s.append(r["outputs2"])
        softs.append(r["softmax_outputs"])
        pres.append(r["pre_output"])
        featps.append(r["features_p"])
    return (np.concatenate(feats, 0), np.concatenate(out2s, 0),
            np.concatenate(softs, 0), np.concatenate(pres, 0),
            np.concatenate(featps, 0))
